# revision 21
# baseline (speedup 1.0000x reference)
"""BiLSTM-CRF forward NLL on 8 Trainium2 NeuronCores (Bass/Tile).

Sharding: data-parallel over batch (B=32 -> BL=4 per core). Each core computes
its 4 samples' log-likelihood pieces; the host does the trivial final
num-logZ / mean reduction when unsharding.

Internal layout: "T-layout" = [feature dim on partitions, rows on free],
rows = t*BL + b (t-major), so a conv time-shift is a free-dim shift by BL
and each LSTM step touches BL adjacent columns.

The LSTM input projections (x @ W_ih^T) run in bf16 (inputs are bounded,
activations saturating; the recurrent path stays fp32). Everything else fp32.
"""

import os
from contextlib import ExitStack

import numpy as np
import ml_dtypes

import concourse.bacc as bacc
import concourse.mybir as mybir
import concourse.tile as tile
from concourse.bass_utils import run_bass_kernel_spmd

F32 = mybir.dt.float32
BF16 = mybir.dt.bfloat16
I32 = mybir.dt.int32
AF = mybir.ActivationFunctionType
ALU = mybir.AluOpType
AX = mybir.AxisListType

V, T, E, H, FD, LLAYERS = 128, 32, 256, 512, 64, 2
B, S_FULL = 32, 512
NH = 4
NCORES = 8
BL = B // NCORES            # 4
D = 2 * H                   # 1024
G = 4 * H                   # 2048
DH = D // NH                # 256
CSH = float(np.log(T))      # per-step log shift for the exp-domain CRF scan


def build_program(S, dbg=False):
    rows = S * BL
    nc = bacc.Bacc("TRN2", target_bir_lowering=False, debug=False,
                   num_devices=NCORES)

    def din(name, shape, dt=F32):
        return nc.dram_tensor(name, shape, dt, kind="ExternalInput")

    # ---- external inputs ----
    featT_d = din("featT", [FD, rows])
    charf_d = din("charf", [1, rows])
    tagsf_d = din("tagsf", [1, rows])
    pairoff_d = din("pairoff", [128, 16], I32)
    seoff_d = din("seoff", [8, 1], I32)
    transflat_d = din("transflat", [T * T + 1, 1])
    sevec_d = din("sevec", [2 * T, 1])

    embed_d = din("embed", [V, E])
    fpwT_d = din("fpwT", [FD, E])
    fpb2_d = din("fpb2", [128, 2])
    cwT_d = din("cwT", [3, 2 * E, E])
    cb2_d = din("cb2", [128, 2])

    wih0T_d = din("wih0T", [2, E, G], BF16)
    whh0T_d = din("whh0T", [2, H, G])
    bias0_d = din("bias0", [2, 1, G])
    wih1T_d = din("wih1T", [2, D, G], BF16)
    whh1T_d = din("whh1T", [2, H, G])
    bias1_d = din("bias1", [2, 1, G])

    wqkvT_d = din("wqkvT", [D, 3 * D])
    bqkvt_d = din("bqkvt", [128, 24])
    bvrow_d = din("bvrow", [1, D])
    woTa_d = din("woTa", [D + 1, D])
    wprimT_d = din("wprimT", [D, T])
    sw_d = din("swv", [T, 1])
    bprim_d = din("bprim", [T, 1])

    transbd_d = din("transbd", [128, 128])
    startbd_d = din("startbd", [128, 1])
    endbd_d = din("endbd", [128, 1])
    onesbd4_d = din("onesbd4", [128, BL])
    onesbd4T_d = din("onesbd4T", [BL, 128])
    eye128_d = din("eye128", [128, 128])
    eye4b_d = din("eye4b", [BL, BL], BF16)
    onesrow_d = din("onesrow", [1, 128])
    onesS_d = din("onesS", [1, S])
    onesp_d = din("onesp", [128, 1])
    iota128_d = din("iota128", [128, 1])

    # ---- outputs ----
    logz_o = nc.dram_tensor("logz_o", [BL, 1], F32, kind="ExternalOutput")
    trsum_o = nc.dram_tensor("trsum_o", [BL, 1], F32, kind="ExternalOutput")
    emsum_o = nc.dram_tensor("emsum_o", [1, BL], F32, kind="ExternalOutput")
    se_o = nc.dram_tensor("se_o", [8, 1], F32, kind="ExternalOutput")
    if dbg:
        xcat_o = nc.dram_tensor("xcat_o", [4 * 128, rows], F32,
                                kind="ExternalOutput")
        conv_o = nc.dram_tensor("conv_o", [2 * 128, rows], F32,
                                kind="ExternalOutput")
        y1_o = nc.dram_tensor("y1_o", [rows, D], F32, kind="ExternalOutput")
        zt_o = nc.dram_tensor("zt_o", [D, rows], F32, kind="ExternalOutput")
        ecrf_o = nc.dram_tensor("ecrf_o", [128, S], F32, kind="ExternalOutput")
        crfdbg_o = nc.dram_tensor("crfdbg_o", [128, 8], F32,
                                  kind="ExternalOutput")

    # ---- internal DRAM scratch ----
    y0t_d = nc.dram_tensor("y0t_scr", [2, 4, 128, rows], BF16)
    y1_d = nc.dram_tensor("y1_scr", [rows, D], F32)
    y1t_d = nc.dram_tensor("y1t_scr", [8, 128, rows], F32)

    def fchunks(n, mx=512):
        o = 0
        while o < n:
            yield o, min(mx, n - o)
            o += mx

    with tile.TileContext(nc) as tc:
        with ExitStack() as top:
            cpool = top.enter_context(tc.tile_pool(name="consts", bufs=1))

            def cload(dram, shape, dt=F32, name=None):
                nm = (name or dram.name) + "_c"
                t_ = cpool.tile(shape, dt, tag=nm, name=nm)
                nc.sync.dma_start(t_[:], dram[:])
                return t_

            eye128 = cload(eye128_d, [128, 128])
            eye4b = cload(eye4b_d, [BL, BL], BF16)
            onesrow = cload(onesrow_d, [1, 128])
            onesS = cload(onesS_d, [1, S])
            onesp = cload(onesp_d, [128, 1])
            iota128 = cload(iota128_d, [128, 1])
            onesbd4 = cload(onesbd4_d, [128, BL])
            onesbd4T = cload(onesbd4T_d, [BL, 128])
            startbd = cload(startbd_d, [128, 1])
            endbd = cload(endbd_d, [128, 1])
            transbd = cload(transbd_d, [128, 128])
            eye4 = eye128[:BL, :BL]
            eps1 = cpool.tile([1, 1], F32, tag="eps1", name="eps1")
            nc.vector.memset(eps1[:], 1e-5)

            # ======== front-end: xcatT = [embedT ; feT] -> conv -> x0T ========
            xst_stack = ExitStack()
            xpool = xst_stack.enter_context(tc.tile_pool(name="x0t", bufs=1))
            x0T = [xpool.tile([128, rows], BF16, tag=f"x0T{c}", name=f"x0T{c}")
                   for c in range(2)]

            fe_stack = ExitStack()
            fpool = fe_stack.enter_context(tc.tile_pool(name="fe", bufs=1))
            fps = fe_stack.enter_context(
                tc.tile_pool(name="fe_ps", bufs=4, space="PSUM"))

            PADR = rows + 2 * BL
            xcat = [fpool.tile([128, PADR], F32, tag=f"xcat{c}", name=f"xcat{c}")
                    for c in range(4)]
            for c in range(4):
                nc.vector.memset(xcat[c][:, 0:BL], 0.0)
                nc.vector.memset(xcat[c][:, BL + rows:PADR], 0.0)

            featT = fpool.tile([FD, rows], F32, tag="featT", name="featT")
            nc.sync.dma_start(featT[:], featT_d[:])
            fpwT = fpool.tile([FD, E], F32, tag="fpwT", name="fpwT")
            nc.sync.dma_start(fpwT[:], fpwT_d[:])
            fpb2 = fpool.tile([128, 2], F32, tag="fpb2", name="fpb2")
            nc.sync.dma_start(fpb2[:], fpb2_d[:])
            embed = fpool.tile([V, E], F32, tag="embed", name="embed")
            nc.sync.dma_start(embed[:], embed_d[:])
            charf = fpool.tile([1, rows], F32, tag="charf", name="charf")
            nc.sync.dma_start(charf[:], charf_d[:])

            ohT = fpool.tile([128, rows], F32, tag="ohT", name="ohT")
            for q0, qn in fchunks(rows):
                ps = fps.tile([128, 512], F32, tag="feps", name="feps")
                nc.tensor.matmul(ps[:, :qn], onesrow[:, :128],
                                 charf[:, q0:q0 + qn], start=True, stop=True)
                nc.vector.tensor_tensor(
                    out=ohT[:, q0:q0 + qn], in0=ps[:, :qn],
                    in1=iota128[:].to_broadcast([128, qn]), op=ALU.is_equal)
            for c in range(2):
                for q0, qn in fchunks(rows):
                    ps = fps.tile([128, 512], F32, tag="feps", name="feps")
                    nc.tensor.matmul(ps[:, :qn], embed[:, 128 * c:128 * (c + 1)],
                                     ohT[:, q0:q0 + qn], start=True, stop=True)
                    nc.scalar.copy(out=xcat[c][:, BL + q0:BL + q0 + qn],
                                   in_=ps[:, :qn])
            for c in range(2):
                for q0, qn in fchunks(rows):
                    ps = fps.tile([128, 512], F32, tag="feps", name="feps")
                    nc.tensor.matmul(ps[:, :qn], fpwT[:, 128 * c:128 * (c + 1)],
                                     featT[:, q0:q0 + qn], start=True, stop=True)
                    nc.scalar.activation(
                        out=xcat[2 + c][:, BL + q0:BL + q0 + qn],
                        in_=ps[:, :qn], func=AF.Relu, bias=fpb2[:, c:c + 1])
            if dbg:
                for c in range(4):
                    nc.sync.dma_start(xcat_o[128 * c:128 * (c + 1), :],
                                      xcat[c][:, BL:BL + rows])

            cwT = fpool.tile([128, 12 * E], F32, tag="cwT", name="cwT")
            for k in range(3):
                for kc in range(4):
                    nc.sync.dma_start(
                        cwT[:, (k * 4 + kc) * E:(k * 4 + kc + 1) * E],
                        cwT_d[k, 128 * kc:128 * (kc + 1), :])
            cb2 = fpool.tile([128, 2], F32, tag="cb2", name="cb2")
            nc.sync.dma_start(cb2[:], cb2_d[:])
            for oc in range(2):
                for q0, qn in fchunks(rows):
                    ps = fps.tile([128, 512], F32, tag="feps", name="feps")
                    first = True
                    for k in range(3):
                        for kc in range(4):
                            lhs = cwT[:, (k * 4 + kc) * E + 128 * oc:
                                      (k * 4 + kc) * E + 128 * (oc + 1)]
                            rhs = xcat[kc][:, BL * k + q0:BL * k + q0 + qn]
                            nc.tensor.matmul(ps[:, :qn], lhs, rhs, start=first,
                                             stop=(k == 2 and kc == 3))
                            first = False
                    nc.scalar.activation(
                        out=x0T[oc][:, q0:q0 + qn], in_=ps[:, :qn],
                        func=AF.Relu, bias=cb2[:, oc:oc + 1])
                    if dbg:
                        dbt = fpool.tile([128, 512], F32, tag="fdbg",
                                         name="fdbg")
                        nc.vector.tensor_copy(out=dbt[:, :qn],
                                              in_=x0T[oc][:, q0:q0 + qn])
                        nc.sync.dma_start(
                            conv_o[128 * oc:128 * (oc + 1), q0:q0 + qn],
                            dbt[:, :qn])
            fe_stack.close()

            # ========================= LSTM =========================
            def lstm_layer(layer):
                lst = ExitStack()
                wpool = lst.enter_context(tc.tile_pool(name=f"lw{layer}", bufs=1))
                xwpool = lst.enter_context(
                    tc.tile_pool(name=f"xw{layer}", bufs=2))
                slot = lst.enter_context(
                    tc.tile_pool(name=f"slot{layer}", bufs=2))
                sv = lst.enter_context(tc.tile_pool(name=f"sv{layer}", bufs=2))
                stream = lst.enter_context(
                    tc.tile_pool(name=f"stream{layer}", bufs=2))
                wstm = lst.enter_context(
                    tc.tile_pool(name=f"wstm{layer}", bufs=8))
                stg = lst.enter_context(tc.tile_pool(name=f"stg{layer}", bufs=2))
                psG = lst.enter_context(
                    tc.tile_pool(name=f"psG{layer}", bufs=1, space="PSUM"))

                in_dim = E if layer == 0 else D
                KCI = in_dim // 128
                whh_dr = whh0T_d if layer == 0 else whh1T_d
                wih_dr = wih0T_d if layer == 0 else wih1T_d
                bias_dr = bias0_d if layer == 0 else bias1_d

                whh, bias_sb = {}, {}
                for d_ in range(2):
                    for kc in range(4):
                        w_ = wpool.tile([128, G], F32, tag=f"whh{d_}_{kc}",
                                        name=f"whh{d_}_{kc}")
                        nc.sync.dma_start(
                            w_[:], whh_dr[d_, 128 * kc:128 * (kc + 1), :])
                        whh[(d_, kc)] = w_
                    b_ = wpool.tile([1, G], F32, tag=f"lb{d_}", name=f"lb{d_}")
                    nc.sync.dma_start(b_[:], bias_dr[d_, :, :])
                    bias_sb[d_] = b_

                c_sb = {d_: wpool.tile([BL, H], F32, tag=f"c{d_}",
                                       name=f"c{d_}") for d_ in range(2)}
                hT_prev, xw_cur, ysl, xst_w = {}, {}, {}, {}

                for t in range(S):
                    for d_ in range(2):
                        w = t // 32
                        r = t % 32
                        tau = t if d_ == 0 else S - 1 - t
                        if r == 0:
                            tau0 = 32 * w if d_ == 0 else S - 32 * (w + 1)
                            xw = xwpool.tile([128, G], BF16, tag=f"xw{d_}",
                                             name=f"xw{d_}")
                            xw_cur[d_] = xw
                            xst_w[d_] = {}
                            if layer == 1:
                                for kc in range(KCI):
                                    sl = stream.tile([128, 128], BF16,
                                                     tag=f"ysl{d_}_{kc}",
                                                     name=f"ysl{d_}_{kc}")
                                    nc.sync.dma_start(
                                        sl[:],
                                        y0t_d[0 if kc < 4 else 1, kc % 4, :,
                                              BL * tau0:BL * tau0 + 128])
                                    ysl[(d_, kc)] = sl
                            for cch in range(4):
                                bps = psG.tile([128, 512], F32,
                                               tag=f"g{d_}{cch}",
                                               name=f"g{d_}{cch}")
                                for kc in range(KCI):
                                    if layer == 0:
                                        lhs = x0T[kc][:, BL * tau0:
                                                      BL * tau0 + 128]
                                    else:
                                        lhs = ysl[(d_, kc)][:]
                                    wsl = wstm.tile([128, 512], BF16,
                                                    tag=f"wsl{d_}",
                                                    name=f"wsl{d_}")
                                    nc.sync.dma_start(
                                        wsl[:],
                                        wih_dr[d_, 128 * kc:128 * (kc + 1),
                                               512 * cch:512 * (cch + 1)])
                                    nc.tensor.matmul(
                                        bps[:], lhs, wsl[:],
                                        start=(kc == 0), stop=False)
                                nc.tensor.matmul(
                                    bps[:], onesrow[:1, :128],
                                    bias_sb[d_][:, 512 * cch:512 * (cch + 1)],
                                    start=False, stop=True)
                                if cch < 2:
                                    nc.scalar.copy(
                                        out=xw[:, 512 * cch:512 * (cch + 1)],
                                        in_=bps[:])
                                else:
                                    nc.vector.tensor_copy(
                                        out=xw[:, 512 * cch:512 * (cch + 1)],
                                        in_=bps[:])
                        xw = xw_cur[d_]
                        rr = r if d_ == 0 else 31 - r
                        if rr in xst_w[d_]:
                            xst = xst_w[d_][rr]
                        else:
                            xst = stg.tile([BL, G], BF16, tag=f"xst{d_}",
                                           name=f"xst{d_}")
                            nc.sync.dma_start(xst[:],
                                              xw[BL * rr:BL * (rr + 1), :])
                            xst_w[d_][rr] = xst
                        gps = []
                        for cch in range(4):
                            gp = psG.tile([BL, 512], F32, tag=f"g{d_}{cch}",
                                          name=f"g{d_}{cch}")
                            gps.append(gp)
                            nc.tensor.matmul(
                                gp[:], eye4b[:],
                                xst[:, 512 * cch:512 * (cch + 1)],
                                start=True, stop=(t == 0))
                            if t > 0:
                                for kc in range(4):
                                    nc.tensor.matmul(
                                        gp[:], hT_prev[(d_, kc)][:],
                                        whh[(d_, kc)][:, 512 * cch:
                                                      512 * (cch + 1)],
                                        start=False, stop=(kc == 3))
                        # gate order i,f,g,o packed in one activation tile
                        ga = sv.tile([BL, G], F32, tag=f"ga{d_}",
                                     name=f"ga{d_}")
                        nc.scalar.activation(out=ga[:, 0:512], in_=gps[0][:],
                                             func=AF.Sigmoid)
                        nc.scalar.activation(out=ga[:, 512:1024], in_=gps[1][:],
                                             func=AF.Sigmoid)
                        nc.scalar.activation(out=ga[:, 1024:1536], in_=gps[2][:],
                                             func=AF.Tanh)
                        nc.scalar.activation(out=ga[:, 1536:2048], in_=gps[3][:],
                                             func=AF.Sigmoid)
                        cs = c_sb[d_]
                        if t == 0:
                            nc.vector.tensor_tensor(out=cs[:], in0=ga[:, 0:512],
                                                    in1=ga[:, 1024:1536],
                                                    op=ALU.mult)
                        else:
                            nc.vector.tensor_tensor(out=cs[:], in0=ga[:, 512:1024],
                                                    in1=cs[:], op=ALU.mult)
                            nc.vector.tensor_tensor(out=ga[:, 0:512],
                                                    in0=ga[:, 0:512],
                                                    in1=ga[:, 1024:1536],
                                                    op=ALU.mult)
                            nc.vector.tensor_tensor(out=cs[:], in0=cs[:],
                                                    in1=ga[:, 0:512],
                                                    op=ALU.add)
                        nc.scalar.activation(out=ga[:, 1024:1536], in_=cs[:],
                                             func=AF.Tanh)
                        h_sb = sv.tile([BL, H], F32, tag=f"h{d_}",
                                       name=f"h{d_}")
                        nc.vector.tensor_tensor(out=h_sb[:],
                                                in0=ga[:, 1536:2048],
                                                in1=ga[:, 1024:1536],
                                                op=ALU.mult)
                        for kc in range(4):
                            tp = psG.tile([128, BL], F32, tag=f"g{d_}{kc}",
                                          name=f"g{d_}{kc}")
                            nc.tensor.transpose(
                                tp[:], h_sb[:, 128 * kc:128 * (kc + 1)], eye4)
                            ht = slot.tile([128, BL], F32, tag=f"ht{d_}_{kc}",
                                           name=f"ht{d_}_{kc}")
                            if kc < 2:
                                nc.scalar.copy(out=ht[:], in_=tp[:])
                            else:
                                nc.vector.tensor_copy(out=ht[:], in_=tp[:])
                            hT_prev[(d_, kc)] = ht
                            if layer == 0:
                                hb = slot.tile([128, BL], BF16,
                                               tag=f"htb{d_}_{kc}",
                                               name=f"htb{d_}_{kc}")
                                nc.gpsimd.tensor_copy(out=hb[:], in_=ht[:])
                                nc.sync.dma_start(
                                    y0t_d[d_, kc, :, BL * tau:BL * (tau + 1)],
                                    hb[:])
                            else:
                                nc.sync.dma_start(
                                    y1t_d[4 * d_ + kc, :,
                                          BL * tau:BL * (tau + 1)], ht[:])
                        if layer == 1:
                            nc.sync.dma_start(
                                y1_d[BL * tau:BL * (tau + 1),
                                     H * d_:H * (d_ + 1)], h_sb[:])
                            if dbg:
                                nc.sync.dma_start(
                                    y1_o[BL * tau:BL * (tau + 1),
                                         H * d_:H * (d_ + 1)], h_sb[:])
                lst.close()

            lstm_layer(0)
            xst_stack.close()
            lstm_layer(1)

            # =============== attention + LN + emissions (per b) ===============
            at = ExitStack()
            apool = at.enter_context(tc.tile_pool(name="attn", bufs=1))
            aps_stack = ExitStack()
            awork = aps_stack.enter_context(tc.tile_pool(name="awork", bufs=2))
            awork1 = aps_stack.enter_context(tc.tile_pool(name="awork1", bufs=1))
            wstr = aps_stack.enter_context(tc.tile_pool(name="wstr", bufs=2))
            wvstr = aps_stack.enter_context(tc.tile_pool(name="wvstr", bufs=1))
            wostr = aps_stack.enter_context(tc.tile_pool(name="wostr", bufs=8))
            aps = aps_stack.enter_context(
                tc.tile_pool(name="aps", bufs=3, space="PSUM"))
            apsS = aps_stack.enter_context(
                tc.tile_pool(name="apsS", bufs=1, space="PSUM"))

            wprimT = [apool.tile([128, T], F32, tag=f"wpr{k}", name=f"wpr{k}")
                      for k in range(8)]
            for k in range(8):
                nc.sync.dma_start(wprimT[k][:],
                                  wprimT_d[128 * k:128 * (k + 1), :])
            bqkvt = apool.tile([128, 24], F32, tag="bqkvt", name="bqkvt")
            nc.sync.dma_start(bqkvt[:], bqkvt_d[:])
            bvrow = apool.tile([1, D], F32, tag="bvrow", name="bvrow")
            nc.sync.dma_start(bvrow[:], bvrow_d[:])
            woTones = apool.tile([1, D], F32, tag="woTones", name="woTones")
            nc.sync.dma_start(woTones[:], woTa_d[D:D + 1, :])
            swv = apool.tile([T, 1], F32, tag="swv", name="swv")
            nc.sync.dma_start(swv[:], sw_d[:])
            bprim = apool.tile([T, 1], F32, tag="bprim", name="bprim")
            nc.sync.dma_start(bprim[:], bprim_d[:])
            tagsrow = apool.tile([1, rows], F32, tag="tagsrow", name="tagsrow")
            nc.sync.dma_start(tagsrow[:], tagsf_d[:])

            ecrf = apool.tile([128, S], F32, tag="ecrf", name="ecrf")
            emsum = apool.tile([1, BL], F32, tag="emsum", name="emsum")
            NT = S // 128

            for b_ in range(BL):
                bst = ExitStack()
                bpool = bst.enter_context(
                    tc.tile_pool(name=f"battn{b_}", bufs=1))
                y1Tb = []
                for k in range(8):
                    yb = bpool.tile([128, S], F32, tag=f"y1tb{k}",
                                    name=f"y1tb{k}")
                    for q0, qn in fchunks(rows):
                        ch = awork.tile([128, 512], F32, tag="y1tch",
                                        name="y1tch")
                        nc.sync.dma_start(ch[:, :qn], y1t_d[k, :, q0:q0 + qn])
                        nc.vector.tensor_copy(
                            out=yb[:, q0 // BL:(q0 + qn) // BL],
                            in_=ch[:, b_:qn:BL])
                    y1Tb.append(yb)

                av_bT = [None] * 8
                for h_ in range(NH):
                    qkT = {0: [], 1: []}
                    for which in (0, 1):
                        base = which * D + h_ * DH
                        for cp in range(2):
                            ps = aps.tile([128, S], F32, tag="bigps",
                                          name="bigps")
                            for k in range(8):
                                wsl = wstr.tile([128, 128], F32, tag="wsl",
                                                name="wsl")
                                nc.sync.dma_start(
                                    wsl[:],
                                    wqkvT_d[128 * k:128 * (k + 1),
                                            base + 128 * cp:
                                            base + 128 * (cp + 1)])
                                nc.tensor.matmul(
                                    ps[:], wsl[:], y1Tb[k][:],
                                    start=(k == 0), stop=(k == 7))
                            sb = awork.tile([128, S], F32, tag=f"qk{which}{cp}",
                                            name=f"qk{which}{cp}")
                            mch = (base + 128 * cp) // 128
                            nc.scalar.activation(
                                out=sb[:], in_=ps[:], func=AF.Identity,
                                bias=bqkvt[:, mch:mch + 1])
                            qkT[which].append(sb)
                    wv_sl = []
                    for k in range(8):
                        wsl = wvstr.tile([128, DH], F32, tag=f"wslv{k}",
                                         name=f"wslv{k}")
                        nc.sync.dma_start(
                            wsl[:], wqkvT_d[128 * k:128 * (k + 1),
                                            2 * D + h_ * DH:
                                            2 * D + (h_ + 1) * DH])
                        wv_sl.append(wsl)
                    v_rows = []
                    for mt in range(NT):
                        ps = aps.tile([128, DH], F32, tag="bigps", name="bigps")
                        for k in range(8):
                            nc.tensor.matmul(
                                ps[:], y1Tb[k][:, 128 * mt:128 * (mt + 1)],
                                wv_sl[k][:], start=(k == 0), stop=False)
                        nc.tensor.matmul(
                            ps[:], onesS[:1, 128 * mt:128 * mt + 128],
                            bvrow[:, h_ * DH:(h_ + 1) * DH],
                            start=False, stop=True)
                        v_sb = awork1.tile([128, DH], F32, tag=f"vr{mt}",
                                           name=f"vr{mt}")
                        nc.vector.tensor_copy(out=v_sb[:], in_=ps[:])
                        v_rows.append(v_sb)
                    expT = []
                    for kt in range(NT):
                        ps = aps.tile([128, S], F32, tag="bigps", name="bigps")
                        for cp in range(2):
                            nc.tensor.matmul(
                                ps[:], qkT[1][cp][:, 128 * kt:128 * (kt + 1)],
                                qkT[0][cp][:], start=(cp == 0), stop=(cp == 1))
                        ex = awork.tile([128, S], F32, tag=f"ex{kt}",
                                        name=f"ex{kt}")
                        nc.scalar.activation(out=ex[:], in_=ps[:], func=AF.Exp,
                                             scale=1.0 / 16.0)
                        expT.append(ex)
                    zp = apsS.tile([1, S], F32, tag="zps", name="zps")
                    for kt in range(NT):
                        nc.tensor.matmul(zp[:], onesp[:, :1], expT[kt][:],
                                         start=(kt == 0), stop=(kt == NT - 1))
                    zinv = awork1.tile([1, S], F32, tag="zinv", name="zinv")
                    nc.vector.reciprocal(zinv[:], zp[:])
                    zr = apsS.tile([128, S], F32, tag="reps", name="reps")
                    nc.tensor.matmul(zr[:], onesrow[:1, :128], zinv[:],
                                     start=True, stop=True)
                    zrep = awork.tile([128, S], F32, tag="zrep", name="zrep")
                    nc.scalar.copy(out=zrep[:], in_=zr[:])
                    for cp in range(2):
                        ps = aps.tile([128, S], F32, tag="bigps", name="bigps")
                        for kt in range(NT):
                            nc.tensor.matmul(
                                ps[:], v_rows[kt][:, 128 * cp:128 * (cp + 1)],
                                expT[kt][:], start=(kt == 0),
                                stop=(kt == NT - 1))
                        av = bpool.tile([128, S], F32,
                                        tag=f"avbt{2 * h_ + cp}",
                                        name=f"avbt{2 * h_ + cp}")
                        nc.vector.tensor_tensor(out=av[:], in0=ps[:],
                                                in1=zrep[:], op=ALU.mult)
                        av_bT[2 * h_ + cp] = av
                # out_proj + residual -> zT_b
                zT_b = []
                for m in range(8):
                    ps = aps.tile([128, S], F32, tag="bigps", name="bigps")
                    for k in range(8):
                        wsl = wostr.tile([128, 128], F32, tag="wo", name="wo")
                        nc.sync.dma_start(
                            wsl[:], woTa_d[128 * k:128 * (k + 1),
                                           128 * m:128 * (m + 1)])
                        nc.tensor.matmul(ps[:], wsl[:], av_bT[k][:],
                                         start=(k == 0), stop=False)
                    nc.tensor.matmul(ps[:], woTones[:, 128 * m:128 * (m + 1)],
                                     onesS[:], start=False, stop=True)
                    zt = bpool.tile([128, S], F32, tag=f"zt{m}", name=f"zt{m}")
                    nc.vector.tensor_tensor(out=zt[:], in0=ps[:],
                                            in1=y1Tb[m][:], op=ALU.add)
                    zT_b.append(zt)
                if dbg:
                    for m in range(8):
                        nc.sync.dma_start(
                            zt_o[128 * m:128 * (m + 1), S * b_:S * (b_ + 1)],
                            zT_b[m][:])
                # LN stats (partition reduction via ones-matmul)
                szp = apsS.tile([1, S], F32, tag="zps", name="zps")
                for m in range(8):
                    nc.tensor.matmul(szp[:], onesp[:, :1], zT_b[m][:],
                                     start=(m == 0), stop=(m == 7))
                szzp = apsS.tile([1, S], F32, tag="zps2", name="zps2")
                for m in range(8):
                    sq = awork.tile([128, S], F32, tag="sqtmp", name="sqtmp")
                    nc.scalar.activation(out=sq[:], in_=zT_b[m][:],
                                         func=AF.Square)
                    nc.tensor.matmul(szzp[:], onesp[:, :1], sq[:],
                                     start=(m == 0), stop=(m == 7))
                mu = awork1.tile([1, S], F32, tag="mu", name="mu")
                nc.vector.tensor_scalar_mul(mu[:], szp[:], 1.0 / D)
                varr = awork1.tile([1, S], F32, tag="varr", name="varr")
                nc.vector.tensor_scalar_mul(varr[:], szzp[:], 1.0 / D)
                musq = awork1.tile([1, S], F32, tag="musq", name="musq")
                nc.vector.tensor_tensor(out=musq[:], in0=mu[:], in1=mu[:],
                                        op=ALU.mult)
                nc.vector.tensor_tensor(out=varr[:], in0=varr[:], in1=musq[:],
                                        op=ALU.subtract)
                sig = awork1.tile([1, S], F32, tag="sig", name="sig")
                nc.scalar.activation(out=sig[:], in_=varr[:], func=AF.Sqrt,
                                     bias=eps1[:, 0:1])
                inv = awork1.tile([1, S], F32, tag="inv", name="inv")
                nc.vector.reciprocal(inv[:], sig[:])
                m2 = awork1.tile([1, S], F32, tag="m2", name="m2")
                nc.vector.tensor_tensor(out=m2[:], in0=mu[:], in1=inv[:],
                                        op=ALU.mult)
                erp = apsS.tile([T, S], F32, tag="erps", name="erps")
                for k in range(8):
                    nc.tensor.matmul(erp[:], wprimT[k][:], zT_b[k][:],
                                     start=(k == 0), stop=(k == 7))
                ivr_p = apsS.tile([T, S], F32, tag="reps", name="reps")
                nc.tensor.matmul(ivr_p[:], onesrow[:1, :T], inv[:],
                                 start=True, stop=True)
                invr = awork1.tile([T, S], F32, tag="invr", name="invr")
                nc.scalar.copy(out=invr[:], in_=ivr_p[:])
                m2r_p = apsS.tile([T, S], F32, tag="reps", name="reps")
                nc.tensor.matmul(m2r_p[:], onesrow[:1, :T], m2[:],
                                 start=True, stop=True)
                m2r = awork1.tile([T, S], F32, tag="m2r", name="m2r")
                nc.scalar.copy(out=m2r[:], in_=m2r_p[:])
                etile = awork1.tile([T, S], F32, tag="etile", name="etile")
                esl = etile[:]
                nc.vector.tensor_tensor(out=esl, in0=erp[:], in1=invr[:],
                                        op=ALU.mult)
                m2s = awork1.tile([T, S], F32, tag="m2s", name="m2s")
                nc.vector.tensor_scalar(out=m2s[:], in0=m2r[:],
                                        scalar1=swv[:, 0:1], scalar2=None,
                                        op0=ALU.mult)
                nc.vector.tensor_tensor(out=esl, in0=esl, in1=m2s[:],
                                        op=ALU.subtract)
                nc.vector.tensor_scalar(out=esl, in0=esl,
                                        scalar1=bprim[:, 0:1], scalar2=None,
                                        op0=ALU.add)
                # em_tag (raw emissions, before CRF shifts)
                ohp = apsS.tile([T, S], F32, tag="reps", name="reps")
                nc.tensor.matmul(ohp[:], onesrow[:1, :T],
                                 tagsrow[:, b_::BL], start=True, stop=True)
                oht = awork1.tile([T, S], F32, tag="oht", name="oht")
                nc.vector.tensor_tensor(
                    out=oht[:], in0=ohp[:],
                    in1=iota128[:T, :].to_broadcast([T, S]), op=ALU.is_equal)
                emt = awork1.tile([T, S], F32, tag="emt", name="emt")
                nc.vector.tensor_tensor(out=emt[:], in0=esl, in1=oht[:],
                                        op=ALU.mult)
                emtp = apsS.tile([1, S], F32, tag="zps2", name="zps2")
                nc.tensor.matmul(emtp[:], onesp[:T, :1], emt[:],
                                 start=True, stop=True)
                nc.vector.reduce_sum(out=emsum[:, b_:b_ + 1], in_=emtp[:],
                                     axis=AX.X)
                # CRF shifts: col0 += start; cols 1.. -= log(T)
                nc.vector.tensor_tensor(out=esl[:, 0:1], in0=esl[:, 0:1],
                                        in1=startbd[0:T, :], op=ALU.add)
                nc.vector.tensor_scalar(out=esl[:, 1:S], in0=esl[:, 1:S],
                                        scalar1=-CSH, scalar2=None, op0=ALU.add)
                # move into the CRF (b,j)-partition layout (DMA can move
                # across partitions; engines cannot)
                nc.sync.dma_start(ecrf[T * b_:T * (b_ + 1), :], etile[:])
                bst.close()
            aps_stack.close()
            if dbg:
                nc.sync.dma_start(ecrf_o[:], ecrf[:])

            # ===================== CRF forward scan =====================
            mbd = cpool.tile([128, 128], F32, tag="mbd", name="mbd")
            nc.scalar.activation(out=mbd[:], in_=transbd[:], func=AF.Exp)
            endexp = cpool.tile([128, 1], F32, tag="endexp", name="endexp")
            nc.scalar.activation(out=endexp[:], in_=endbd[:], func=AF.Exp)
            Em = apool.tile([128, S], F32, tag="Em", name="Em")
            nc.scalar.activation(out=Em[:], in_=ecrf[:], func=AF.Exp)

            crfps = at.enter_context(
                tc.tile_pool(name="crfps", bufs=2, space="PSUM"))
            crfsb = at.enter_context(tc.tile_pool(name="crfsb", bufs=2))
            if dbg:
                crfdbg = apool.tile([128, 8], F32, tag="crfdbg", name="crfdbg")
                nc.vector.tensor_copy(out=crfdbg[:, 6:7], in_=Em[:, 0:1])
                nc.vector.tensor_copy(out=crfdbg[:, 7:8], in_=mbd[:, 0:1])
            logacc = crfsb.tile([BL, 1], F32, tag="lacc", name="lacc",
                                bufs=1)
            nc.vector.memset(logacc[:], 0.0)
            v_prev = Em[:, 0:1]
            for t in range(1, S):
                u = crfps.tile([128, 1], F32, tag="u", name="u")
                nc.tensor.matmul(u[:], mbd[:], v_prev, start=True, stop=True)
                v = crfsb.tile([128, 1], F32, tag="v", name="v")
                nc.vector.tensor_tensor(out=v[:], in0=u[:],
                                        in1=Em[:, t:t + 1], op=ALU.mult)
                v_prev = v[:]
                if t % 16 == 0 and t < S - 1:
                    # renormalize (exp-domain values grow ~e^1/step): divide v
                    # by its per-sample sum, accumulate log of the scale
                    z4r = crfps.tile([BL, 1], F32, tag="z4", name="z4")
                    nc.tensor.matmul(z4r[:], onesbd4[:], v[:],
                                     start=True, stop=True)
                    lr = crfsb.tile([BL, 1], F32, tag="lr", name="lr")
                    nc.scalar.activation(out=lr[:], in_=z4r[:], func=AF.Ln)
                    nc.vector.tensor_tensor(out=logacc[:], in0=logacc[:],
                                            in1=lr[:], op=ALU.add)
                    r4 = crfsb.tile([BL, 1], F32, tag="r4", name="r4")
                    nc.vector.reciprocal(r4[:], z4r[:])
                    vsc = crfps.tile([128, 1], F32, tag="vsc", name="vsc")
                    nc.tensor.matmul(vsc[:], onesbd4T[:], r4[:],
                                     start=True, stop=True)
                    vn = crfsb.tile([128, 1], F32, tag="v", name="v")
                    nc.vector.tensor_tensor(out=vn[:], in0=vsc[:],
                                            in1=v[:], op=ALU.mult)
                    v_prev = vn[:]
                if dbg and t in (1, 2, 16, S - 1):
                    idx = {1: 0, 2: 1, 16: 2, S - 1: 3}[t]
                    nc.vector.tensor_copy(out=crfdbg[:, idx:idx + 1], in_=v[:])
            wend = crfsb.tile([128, 1], F32, tag="wend", name="wend")
            nc.vector.tensor_tensor(out=wend[:], in0=v_prev, in1=endexp[:],
                                    op=ALU.mult)
            z4 = crfps.tile([BL, 1], F32, tag="z4", name="z4")
            nc.tensor.matmul(z4[:], onesbd4[:], wend[:], start=True, stop=True)
            lz = crfsb.tile([BL, 1], F32, tag="lz", name="lz")
            nc.scalar.activation(out=lz[:], in_=z4[:], func=AF.Ln)
            nc.vector.tensor_tensor(out=lz[:], in0=lz[:], in1=logacc[:],
                                    op=ALU.add)
            nc.vector.tensor_scalar(out=lz[:], in0=lz[:],
                                    scalar1=float((S - 1) * CSH), scalar2=None,
                                    op0=ALU.add)
            if dbg:
                nc.vector.tensor_copy(out=crfdbg[:, 4:5], in_=wend[:])
                nc.vector.tensor_copy(out=crfdbg[:, 5:6], in_=endexp[:])
                nc.sync.dma_start(crfdbg_o[:], crfdbg[:])
            nc.sync.dma_start(logz_o[:], lz[:])
            nc.sync.dma_start(emsum_o[:], emsum[:])

            # ===================== numerator gathers =====================
            gpool = at.enter_context(tc.tile_pool(name="gat", bufs=1))
            pairoff = gpool.tile([128, 16], I32, tag="pairoff", name="pairoff")
            nc.sync.dma_start(pairoff[:], pairoff_d[:])
            seoff = gpool.tile([8, 1], I32, tag="seoff", name="seoff")
            nc.sync.dma_start(seoff[:], seoff_d[:])
            trg = gpool.tile([128, 16], F32, tag="trg", name="trg")
            for c_ in range(16):
                nc.gpsimd.indirect_dma_start(
                    out=trg[:, c_:c_ + 1], out_offset=None,
                    in_=transflat_d[:],
                    in_offset=_ioa(pairoff[:, c_:c_ + 1]))
            trs = gpool.tile([128, 1], F32, tag="trs", name="trs")
            nc.vector.reduce_sum(out=trs[:], in_=trg[:], axis=AX.X)
            trp = crfps.tile([BL, 1], F32, tag="z4", name="z4")
            nc.tensor.matmul(trp[:], onesbd4[:], trs[:], start=True, stop=True)
            trsb = gpool.tile([BL, 1], F32, tag="trsb", name="trsb")
            nc.vector.tensor_copy(out=trsb[:], in_=trp[:])
            nc.sync.dma_start(trsum_o[:], trsb[:])
            seg = gpool.tile([8, 1], F32, tag="seg", name="seg")
            nc.gpsimd.indirect_dma_start(
                out=seg[:], out_offset=None, in_=sevec_d[:],
                in_offset=_ioa(seoff[:, 0:1]))
            nc.sync.dma_start(se_o[:], seg[:])
            at.close()
    nc.compile()
    return nc


def _ioa(ap):
    import concourse.bass as bass
    return bass.IndirectOffsetOnAxis(ap=ap, axis=0)


_PROG_CACHE = {}


def _get_prog(S, dbg=False):
    key = (S, dbg)
    if key not in _PROG_CACHE:
        _PROG_CACHE[key] = build_program(S, dbg)
    return _PROG_CACHE[key]


def host_prep(inputs, S, core):
    f32 = lambda x: np.ascontiguousarray(np.asarray(x), dtype=np.float32)
    b0 = core * BL
    feats = f32(inputs["features"])[b0:b0 + BL, :S]
    char = np.asarray(inputs["char_indices"])[b0:b0 + BL, :S]
    tags = np.asarray(inputs["tags"])[b0:b0 + BL, :S].astype(np.int64)

    featT = f32(feats.transpose(2, 1, 0).reshape(FD, S * BL))
    charf = f32(char.T.reshape(1, S * BL))
    tagsf = f32(tags.T.reshape(1, S * BL))

    pairoff = np.full((128, 16), T * T, dtype=np.int32)
    for b in range(BL):
        for q in range(T):
            for f in range(16):
                i = q * 16 + f
                if i < S - 1:
                    pairoff[b * T + q, f] = tags[b, i] * T + tags[b, i + 1]
    seoff = np.zeros((8, 1), dtype=np.int32)
    for b in range(BL):
        seoff[b, 0] = tags[b, 0]
        seoff[4 + b, 0] = T + tags[b, S - 1]

    return {"featT": featT, "charf": charf, "tagsf": tagsf,
            "pairoff": pairoff, "seoff": seoff}


def host_prep_shared(inputs, S):
    f32 = lambda x: np.ascontiguousarray(np.asarray(x), dtype=np.float32)
    bf16 = lambda x: np.ascontiguousarray(
        np.asarray(x, dtype=np.float32)).astype(ml_dtypes.bfloat16)
    d = {}
    d["embed"] = f32(inputs["embed_table"])
    bnscale = f32(inputs["bn_g"]) / np.sqrt(np.float32(1.0 + 1e-5))
    fp_w = f32(inputs["fp_w"]) * bnscale[:, None]
    fp_b = f32(inputs["fp_b"]) * bnscale + f32(inputs["bn_b"])
    d["fpwT"] = f32(fp_w.T)
    d["fpb2"] = f32(fp_b.reshape(2, 128).T)
    cw = f32(inputs["conv_w"])
    d["cwT"] = f32(cw.transpose(2, 1, 0))
    d["cb2"] = f32(f32(inputs["conv_b"]).reshape(2, 128).T)

    lstm = inputs["lstm_params"]
    for layer in range(2):
        wihT = np.stack([f32(lstm[layer][dd][0]).T for dd in range(2)])
        whhT = np.stack([f32(lstm[layer][dd][1]).T for dd in range(2)])
        bias = np.stack([(f32(lstm[layer][dd][2]) +
                          f32(lstm[layer][dd][3]))[None, :]
                         for dd in range(2)])
        d[f"wih{layer}T"] = bf16(wihT)
        d[f"whh{layer}T"] = whhT
        d[f"bias{layer}"] = bias

    d["wqkvT"] = f32(f32(inputs["in_proj_w"]).T)
    bqkv = f32(inputs["in_proj_b"])
    d["bqkvt"] = f32(bqkv.reshape(24, 128).T)
    d["bvrow"] = f32(bqkv[2 * D:][None, :])
    woT = f32(inputs["out_proj_w"]).T
    d["woTa"] = f32(np.concatenate([woT, f32(inputs["out_proj_b"])[None, :]]))
    wprim = f32(inputs["h2t_w"]) * f32(inputs["ln_g"])[None, :]
    d["wprimT"] = f32(wprim.T)
    d["swv"] = f32(wprim.sum(1)[:, None])
    d["bprim"] = f32(
        (f32(inputs["h2t_b"]) +
         f32(inputs["h2t_w"]) @ f32(inputs["ln_b"]))[:, None])

    trans = f32(inputs["crf_trans"])
    tbd = np.full((128, 128), -60.0, dtype=np.float32)
    for b in range(BL):
        tbd[T * b:T * (b + 1), T * b:T * (b + 1)] = trans
    d["transbd"] = tbd
    d["startbd"] = f32(np.tile(f32(inputs["crf_start"]), BL)[:, None])
    d["endbd"] = f32(np.tile(f32(inputs["crf_end"]), BL)[:, None])
    d["transflat"] = f32(np.concatenate([trans.reshape(-1), [0.0]])[:, None])
    d["sevec"] = f32(np.concatenate([f32(inputs["crf_start"]),
                                     f32(inputs["crf_end"])])[:, None])

    obd = np.zeros((128, BL), dtype=np.float32)
    for b in range(BL):
        obd[T * b:T * (b + 1), b] = 1.0
    d["onesbd4"] = obd
    d["onesbd4T"] = np.ascontiguousarray(obd.T)
    d["eye128"] = np.eye(128, dtype=np.float32)
    d["eye4b"] = np.eye(BL, dtype=np.float32).astype(ml_dtypes.bfloat16)
    d["onesrow"] = np.ones((1, 128), dtype=np.float32)
    d["onesS"] = np.ones((1, S), dtype=np.float32)
    d["onesp"] = np.ones((128, 1), dtype=np.float32)
    d["iota128"] = np.arange(128, dtype=np.float32)[:, None]
    return d


def run_cores(inputs, S, dbg=False):
    nc = _get_prog(S, dbg)
    shared = host_prep_shared(inputs, S)
    in_maps = []
    for core in range(NCORES):
        m = dict(shared)
        m.update(host_prep(inputs, S, core))
        in_maps.append(m)
    res = run_bass_kernel_spmd(nc, in_maps, list(range(NCORES)))
    return res.results


def kernel(**inputs) -> np.ndarray:
    S = np.asarray(inputs["char_indices"]).shape[1]
    results = run_cores(inputs, S)
    llh = []
    for core in range(NCORES):
        r = results[core]
        emsum = r["emsum_o"].reshape(BL)
        trsum = r["trsum_o"].reshape(BL)
        se = r["se_o"].reshape(8)
        logz = r["logz_o"].reshape(BL)
        num = emsum + trsum + se[:BL] + se[BL:]
        llh.append(num - logz)
    llh = np.concatenate(llh)
    return np.float32(-llh.mean())
